# revision 8
# baseline (speedup 1.0000x reference)
"""Multi-head attention block kernel for Trainium2 (8 NeuronCores). v5

Changes vs v2 (151.8us), measured 127.1us loop-slope on this hardware:
  - Q/K projection GEMM in fp8e4 DoubleRow (K=256/instruction, 2x PE
    throughput): W_qkv q|k columns are quantized to fp8 at 2048x in the
    prologue, x^T gets an fp8 shadow copy (DVE), and the 2048^2 descale
    folds into the exp scale with the q/k bias pre-scaled by 2048.
  - attention P@V in fp8e4 DoubleRow over key-block pairs: exp writes
    probabilities directly as fp8 (range [e^-2.5, e^2.5] is all-normal
    in e4m3), v_sb is stored fp8 by the existing PSUM->SBUF copies.
  - scores (K=128: DoubleRow needs K>=256), V projection and the output
    GEMM stay bf16: fp8 there pushes rel_err past the 2e-2 gate.
  - the output GEMM runs one iteration DEFERRED: its 8x8 kh-matmuls
    sprinkle one-per-j through the next iteration's exp-paced phases as
    PE gap filler (the exp stream paces each head phase at ~8.3us vs
    ~7.1us of PE work), and the old 16us ACT-idle tail disappears.  The
    For_i body is emitted twice with outT buffers swapped; an epilogue
    computes the last iteration's output GEMM for real, so the
    single-shot result is exact.
  - x^T fp8 casts and half the v_sb copies run on ACT (idle until the
    first exp) instead of DVE, whose body-start queue otherwise delays
    the paced qk-bias and attention-normalize chains.
"""

import numpy as np

NH = 8
SKIP_ATTN = False
SKIP_EXP = False
AT8 = True
SIMDR = False
JGRAN = 1
PSBIG = 2
PSBANK = 4

P = 128
N_CTX = 1024
DIM = 512
H = 8
HD = 128
QKV = 3072
SCALE = 0.125  # (512 // 8) ** -0.5, faithful to the reference
WS = 2048.0  # W_qkv q|k-column fp8 quantize scale; q,k carry it into bf16
ESCALE = SCALE / (WS * WS)  # exp scale absorbs both 2048 factors

_cached_nc = None


def _build_nc(loop_n=1):
    from contextlib import ExitStack

    import concourse.mybir as mybir
    import concourse.tile as tile
    from concourse import bacc
    from concourse.masks import make_identity

    F32 = mybir.dt.float32
    BF16 = mybir.dt.bfloat16
    FP8 = mybir.dt.float8e4
    DR = mybir.MatmulPerfMode.DoubleRow
    AF = mybir.ActivationFunctionType
    ADD = mybir.AluOpType.add
    MULT = mybir.AluOpType.mult

    nc = bacc.Bacc()

    x_ext = nc.declare_dram_parameter("x", [N_CTX, DIM], F32, isOutput=False)
    wqkv_ext = nc.declare_dram_parameter("W_qkv", [DIM, QKV], F32, isOutput=False)
    bqkv_ext = nc.declare_dram_parameter("b_qkv", [QKV], F32, isOutput=False)
    wout_ext = nc.declare_dram_parameter("W_out", [N_CTX, DIM], F32, isOutput=False)
    bout_ext = nc.declare_dram_parameter("b_out", [DIM], F32, isOutput=False)
    out_ext = nc.declare_dram_parameter("out", [N_CTX, DIM], F32, isOutput=True)

    NT = N_CTX // P  # 8 row tiles
    KD = DIM // P  # 4 contraction chunks for dim=512
    VW = HD + 1  # 129: v columns per head incl. ones column

    with ExitStack() as ctx:
        tc = ctx.enter_context(tile.TileContext(nc))
        consts = ctx.enter_context(tc.tile_pool(name="consts", bufs=1))
        persist = ctx.enter_context(tc.tile_pool(name="persist", bufs=1))
        work = ctx.enter_context(tc.tile_pool(name="work", bufs=2))
        small = ctx.enter_context(tc.tile_pool(name="small", bufs=3))
        ps_big = ctx.enter_context(tc.tile_pool(name="ps_big", bufs=PSBIG, space="PSUM"))
        ps_bank = ctx.enter_context(tc.tile_pool(name="ps_bank", bufs=PSBANK, space="PSUM"))

        # ---- constants / weights (outside any bench loop) -------------------
        ident = consts.tile([P, P], BF16, tag="ident")
        make_identity(nc, ident)
        ones_row = consts.tile([1, P], BF16, tag="ones_row")
        nc.vector.memset(ones_row, 1.0)

        # x via HWDGE fp32 (sync queue), cast + PE-transposed on chip —
        # keeps the gpsimd (SWDGE cast) queue free for the weight loads
        x_sb = persist.tile([P, NT, DIM], F32, tag="x_sb")
        for t in range(NT):
            nc.sync.dma_start(
                x_sb[:, t, :], x_ext.rearrange("(t p) d -> p t d", p=P)[:, t, :]
            )

        # v bias and out bias as single-partition rows (bf16, for the
        # prologue broadcast matmuls).  These go FIRST on the gpsimd queue
        # so the broadcast matmuls (head of the PE stream) unblock fast.
        bv_row = consts.tile([1, H * HD], BF16, tag="bv")
        nc.gpsimd.dma_start(bv_row, bqkv_ext[2 * H * P : QKV][None, :])
        bout_row = consts.tile([1, DIM], BF16, tag="bout")
        nc.gpsimd.dma_start(bout_row, bout_ext[None, :])
        # V columns of W_qkv as (p, ko, 1024) bf16 — the V GEMM runs first.
        wv_sb = consts.tile([P, KD, H * HD], BF16, tag="wv")
        wq_r = wqkv_ext.rearrange("(ko p) n -> p ko n", p=P)
        for k in range(KD):
            nc.gpsimd.dma_start(wv_sb[:, k, :], wq_r[:, k, 2 * H * P :])
        # q|k columns staged f32 per k-chunk through a rotating 2-buf pool
        # (8 KB/partition live instead of 32), DVE-quantized to fp8 at 2048x
        # (Pool's software fp8 convert is ~4x slower than DVE's).
        wqk8 = consts.tile([P, KD, 2 * H * P], FP8, tag="wqk8")
        with tc.tile_pool(name="stage", bufs=2) as stage:
            for k in range(KD):
                wqk_st = stage.tile([P, 2 * H * P], F32, tag="wqk_st")
                nc.sync.dma_start(wqk_st, wq_r[:, k, 0 : 2 * H * P])
                nc.vector.tensor_scalar(wqk8[:, k, :], wqk_st, WS, None, MULT)
        # W_out as (p, kh, 512) bf16 — contraction dim (h*hd) on partitions
        wout_sb = consts.tile([P, H, DIM], BF16, tag="wout")
        nc.gpsimd.dma_start(wout_sb, wout_ext.rearrange("(kh p) c -> p kh c", p=P))
        # q/k bias in partition-major layout: bqk[p, m] = b_qkv[m*128 + p],
        # pre-scaled by 2048 to match the fp8 q/k psum scale
        bqk_st = consts.tile([P, 2 * H], F32, tag="bqk_st")
        nc.sync.dma_start(
            bqk_st, bqkv_ext[0 : 2 * H * P].rearrange("(t p) -> p t", p=P)
        )
        bqk_sb = consts.tile([P, 2 * H], F32, tag="bqk")
        nc.vector.tensor_scalar(bqk_sb, bqk_st, WS, None, MULT)

        # ---- prologue: broadcast bias rows across all 128 partitions -------
        # bc[p, c] = bias[c] via ones-column (K=1) matmuls, once.
        bc_bv = consts.tile([P, H * HD], BF16, tag="bc_bv")
        bc_bout = consts.tile([P, DIM], F32, tag="bc_bout")
        ones_col = consts.tile([1, P], BF16, tag="ones_col")
        nc.vector.memset(ones_col, 1.0)
        for half in range(2):
            bps = ps_bank.tile([P, DIM], F32, tag="bank")
            nc.tensor.matmul(
                bps,
                ones_col,
                bv_row[:, half * DIM : (half + 1) * DIM],
                start=True,
                stop=True,
            )
            nc.vector.tensor_copy(bc_bv[:, half * DIM : (half + 1) * DIM], bps)
        bps = ps_bank.tile([P, DIM], F32, tag="bank")
        nc.tensor.matmul(bps, ones_col, bout_row, start=True, stop=True)
        nc.vector.tensor_copy(bc_bout, bps)

        # out staging lives outside the loop: the last two chunks' DRAM
        # stores are deferred into the NEXT iteration (the For_i drain
        # otherwise idles the PE ~3us waiting on the last store's DMA
        # semaphore); an epilogue after the loop stores them for real.
        out_sb = persist.tile([P, NT, DIM], F32, tag="out_sb")
        out_r = out_ext.rearrange("(t p) c -> p t c", p=P)
        nc.vector.memset(out_sb[:, NT - 2 :, :], 0.0)
        # two outT buffers: the output GEMM for iteration i runs one
        # iteration DEFERRED, sprinkled through iteration i+1's exp-paced
        # phases as PE filler (one kh-matmul per j slot).  The For_i body
        # is emitted twice with the buffers swapped; an epilogue finishes
        # the last iteration's output GEMM for real.
        outT_a = persist.tile([P, H, N_CTX], BF16, tag="outT_a")
        outT_b = persist.tile([P, H, N_CTX], BF16, tag="outT_b")
        nc.vector.memset(outT_a, 0.0)
        nc.vector.memset(outT_b, 0.0)

        def body(outT, outT_prev):
            # ---- x^T: cast to bf16 on Pool, transpose via DMA XBAR on the
            # ACT hwdge ring (separate from the SP ring doing out stores).
            # One batched DMA per row-tile: [128, 512] -> 3D out [128, 4, 128].
            x_bf = work.tile([P, NT, DIM], BF16, tag="x_bf")
            for t in range(NT):
                nc.gpsimd.tensor_copy(x_bf[:, t, :], x_sb[:, t, :])
            xT = work.tile([P, KD, N_CTX], BF16, tag="xT")
            # first two row-tiles transposed on the PE (idle at body start,
            # and ~3us faster than the DMA chain's first-byte latency) so the
            # V GEMM starts immediately; the rest stream via DMA XBAR.
            PE_T = 2
            for t in range(PE_T):
                for c in range(KD):
                    tp = ps_bank.tile([P, P], F32, tag="bank")
                    nc.tensor.matmul(
                        tp,
                        x_bf[:, t, c * P : (c + 1) * P],
                        ident,
                        start=True,
                        stop=True,
                    )
                    nc.vector.tensor_copy(xT[:, c, t * P : (t + 1) * P], tp)
            for t in range(PE_T, NT):
                nc.scalar.dma_start(
                    xT[:, :, t * P : (t + 1) * P],
                    x_bf[:, t, :],
                    transpose=True,
                )
            # fp8 shadow of x^T for the q/k DoubleRow GEMM.  ACT does the
            # convert: it idles until the first exp anyway, while DVE's
            # body-start queue feeds the paced qk-bias/attention chains.
            x8T = work.tile([P, KD, N_CTX], FP8, tag="x8T")
            for t in range(NT):
                nc.scalar.copy(
                    x8T[:, :, t * P : (t + 1) * P], xT[:, :, t * P : (t + 1) * P]
                )

            # ---- v (Form A): n on partitions, heads side by side with a
            # ones column: v_sb[:, t, h*129+128] = 1.0 -> softmax sums ride
            # along in the attention matmul for free.  Emission of the V
            # chunks is deferred into phase 0's j-loop (emit_v below) so the
            # free-running V GEMM fills the PE while exp paces the scores.
            v_sb = work.tile([P, NT, H * VW], FP8 if AT8 else BF16, tag="v_sb")
            nc.vector.memset(
                v_sb.rearrange("p t (h w) -> p t h w", w=VW)[:, :, :, HD : HD + 1],
                1.0,
            )

            def emit_v(t, half):
                ps = ps_bank.tile([P, DIM], F32, tag="bank")
                for k in range(KD):
                    nc.tensor.matmul(
                        ps,
                        xT[:, k, t * P : (t + 1) * P],
                        wv_sb[:, k, half * DIM : (half + 1) * DIM],
                        start=(k == 0),
                        stop=(k == KD - 1),
                    )
                dst = v_sb[:, t, :].rearrange("p (h w) -> p h w", w=VW)[
                    :, half * 4 : (half + 1) * 4, 0:HD
                ]
                src = ps.rearrange("p (h w) -> p h w", w=HD)
                # v stays UNbiased: P@(v+bv)/d == (P@v)/d + bv, so bv is
                # added during the per-query normalize instead.  Half the
                # copies go to ACT to unclog DVE's phase-0 queue.
                if half == 0:
                    nc.vector.tensor_copy(dst, src)
                else:
                    nc.scalar.copy(dst, src)

            emit_v(0, 0)
            emit_v(0, 1)

            # ---- per-head software pipeline --------------------------------
            # Engines execute their scheduled streams in-order, so the
            # EMISSION order is the schedule.  Interleave head h's scores
            # (whose PSUM slots recycle at ScalarE's exp pace) with head
            # h-1's attention matmuls so the PE never waits inline on exp;
            # the final GEMM interleaves with the last head's attention.
            def emit_qk(h):
                pair = []
                for part in range(2):  # 0: q, 1: k
                    m = part * H + h
                    qk = work.tile([P, N_CTX], BF16, tag=f"qkT{part}")
                    for half in range(2):
                        sl = slice(half * DIM, (half + 1) * DIM)
                        ps = ps_bank.tile([P, DIM], F32, tag="bank")
                        for kp in range(2):
                            if SIMDR:
                                nc.tensor.matmul(
                                    ps,
                                    wqk8[:, 2 * kp, m * P : (m + 1) * P],
                                    x8T[:, 2 * kp, sl],
                                    start=(kp == 0),
                                    stop=(kp == 1),
                                )
                            else:
                                nc.tensor.matmul(
                                    ps,
                                    wqk8[:, 2 * kp : 2 * kp + 2, m * P : (m + 1) * P],
                                    x8T[:, 2 * kp : 2 * kp + 2, sl],
                                    start=(kp == 0),
                                    stop=(kp == 1),
                                    perf_mode=DR,
                                )
                        nc.vector.tensor_scalar_add(
                            qk[:, sl], ps, bqk_sb[:, m : m + 1]
                        )
                    pair.append(qk)
                return pair

            def emit_scores_j(qkT_pair, pT, j):
                qT_h, kT_h = qkT_pair
                ps = ps_big.tile([P, N_CTX], F32, tag="big")
                for half in range(2):
                    sl = slice(half * DIM, (half + 1) * DIM)
                    nc.tensor.matmul(
                        ps[:, sl],
                        kT_h[:, j * P : (j + 1) * P],
                        qT_h[:, sl],
                        start=True,
                        stop=True,
                    )
                if not SKIP_EXP:
                    nc.scalar.activation(pT[:, j, :], ps, AF.Exp, scale=ESCALE)
                else:
                    nc.vector.tensor_copy(pT[:, j, 0:4], ps[:, 0:4])

            def emit_attn_ic(h, pT, ic, at_stage):
                if SKIP_ATTN:
                    nc.vector.memset(at_stage[:, ic, :], 0.5)
                    return
                aps = ps_bank.tile([P, VW], F32, tag="bank")
                if AT8 and SIMDR:
                    for j in range(0, NT, 2):
                        nc.tensor.matmul(
                            aps[:, :VW],
                            pT[:, j, ic * P : (ic + 1) * P],
                            v_sb[:, j, h * VW : (h + 1) * VW],
                            start=(j == 0),
                            stop=(j == NT - 2),
                        )
                elif AT8:
                    for j in range(0, NT, 2):
                        nc.tensor.matmul(
                            aps[:, :VW],
                            pT[:, j : j + 2, ic * P : (ic + 1) * P],
                            v_sb[:, j : j + 2, h * VW : (h + 1) * VW],
                            start=(j == 0),
                            stop=(j == NT - 2),
                            perf_mode=DR,
                        )
                else:
                    for j in range(NT):
                        nc.tensor.matmul(
                            aps[:, :VW],
                            pT[:, j, ic * P : (ic + 1) * P],
                            v_sb[:, j, h * VW : (h + 1) * VW],
                            start=(j == 0),
                            stop=(j == NT - 1),
                        )
                rc = small.tile([P, 1], F32, tag="rc")
                nc.vector.reciprocal(rc, aps[:, HD : HD + 1])
                # at = (P@v)*1/d + bv  (fused normalize + v-bias)
                nc.vector.scalar_tensor_tensor(
                    at_stage[:, ic, :],
                    aps[:, 0:HD],
                    rc,
                    bc_bv[:, h * HD : (h + 1) * HD],
                    op0=MULT,
                    op1=ADD,
                )

            def emit_transp_dma(h, at_stage):
                # one batched XBAR transpose for the whole head, on the SP
                # HWDGE ring: a 667ns queue op on the ACT sequencer would
                # sit between exp activations and stall the paced stream.
                nc.sync.dma_start(
                    outT[:, h, :].rearrange("p (g q) -> p g q", q=P),
                    at_stage,
                    transpose=True,
                )

            def emit_transp_pe(h, ic, at_stage):
                tp = ps_bank.tile([P, P], F32, tag="bank")
                nc.tensor.matmul(
                    tp, at_stage[:, ic, :], ident, start=True, stop=True
                )
                nc.vector.tensor_copy(outT[:, h, ic * P : (ic + 1) * P], tp)

            fin_state = {}

            def emit_final_kh(ic, kh):
                # one kh-step of the deferred output GEMM for chunk ic of
                # the PREVIOUS iteration; the psum bank stays live across
                # the phase's j-loop and drains via stt+store at kh==H-1.
                if kh == 0:
                    fps_new = ps_bank.tile([P, DIM], F32, tag="bank")
                    fin_state[ic] = fps_new
                fps = fin_state[ic]
                nc.tensor.matmul(
                    fps,
                    outT_prev[:, kh, ic * P : (ic + 1) * P],
                    wout_sb[:, kh, :],
                    start=(kh == 0),
                    stop=(kh == H - 1),
                )
                if kh == H - 1:
                    # out = fps*1 + b_out  (fused copy + bias on DVE)
                    nc.vector.scalar_tensor_tensor(
                        out_sb[:, ic, :], fps, 1.0, bc_bout, op0=MULT, op1=ADD
                    )
                    nc.sync.dma_start(out_r[:, ic, :], out_sb[:, ic, :])

            # qk for head h is emitted mid-phase h-1 (right after scores
            # j=2): its DVE bias adds drain ahead of the attention stt
            # backlog, so the next head's scores (and the paced exp stream)
            # start without the ~2us head-boundary DVE latency bubble.
            qk_use = emit_qk(0)
            pT_prev = work.tile([P, NT, N_CTX], FP8 if AT8 else BF16, tag="pT")
            v_rest = [(t, half) for t in range(1, NT) for half in range(2)]
            qk_next = None
            for j in range(NT):
                emit_scores_j(qk_use, pT_prev, j)
                if j == 2 and NH > 1:
                    qk_next = emit_qk(1)
                # V GEMM chunks fill the exp-paced gaps of phase 0
                for _ in range(2):
                    if v_rest:
                        emit_v(*v_rest.pop(0))
            for h in range(1, NH + 1):
                hp = h - 1  # head whose attention runs this phase
                if h < NH:
                    qk_cur, qk_next = qk_next, None
                    pT_cur = work.tile([P, NT, N_CTX], FP8 if AT8 else BF16, tag="pT")
                at_stage = work.tile([P, NT, P], BF16, tag="at_stage")
                for j in range(NT):
                    if h < NH:
                        emit_scores_j(qk_cur, pT_cur, j)
                        if j == 2 and h + 1 < NH:
                            qk_next = emit_qk(h + 1)
                    # attention of the previous head fills the exp latency
                    emit_attn_ic(hp, pT_prev, j, at_stage)
                    # one kh-step of the deferred output GEMM per j slot
                    emit_final_kh(h - 1, j)
                # whole-head batched XBAR transpose — with the output GEMM
                # deferred a full iteration, even the last head's outT isn't
                # needed until the next body, so the DMA path (instead of
                # PE matmuls + DVE copies) works for all 8 heads.
                emit_transp_dma(hp, at_stage)
                if h < NH:
                    pT_prev = pT_cur

        def epilogue(outT_last):
            # the last iteration's output GEMM, done for real
            for ic in range(NT):
                fps = ps_bank.tile([P, DIM], F32, tag="bank")
                for kh in range(H):
                    nc.tensor.matmul(
                        fps,
                        outT_last[:, kh, ic * P : (ic + 1) * P],
                        wout_sb[:, kh, :],
                        start=(kh == 0),
                        stop=(kh == H - 1),
                    )
                nc.vector.scalar_tensor_tensor(
                    out_sb[:, ic, :], fps, 1.0, bc_bout, op0=MULT, op1=ADD
                )
                nc.sync.dma_start(out_r[:, ic, :], out_sb[:, ic, :])

        if loop_n == 1:
            body(outT_a, outT_b)
        else:
            with tc.For_i(0, max(1, loop_n // 2), 1):
                body(outT_a, outT_b)
                body(outT_b, outT_a)
        epilogue(outT_a)

    nc.finalize()
    return nc


def _get_nc():
    global _cached_nc
    if _cached_nc is None:
        _cached_nc = _build_nc()
    return _cached_nc


def kernel(**inputs):
    from concourse.bass_utils import run_bass_kernel_spmd

    x = np.ascontiguousarray(np.asarray(inputs["x"], dtype=np.float32))
    W_qkv = np.ascontiguousarray(np.asarray(inputs["W_qkv"], dtype=np.float32))
    b_qkv = np.ascontiguousarray(np.asarray(inputs["b_qkv"], dtype=np.float32))
    W_out = np.ascontiguousarray(np.asarray(inputs["W_out"], dtype=np.float32))
    b_out = np.ascontiguousarray(np.asarray(inputs["b_out"], dtype=np.float32))

    bt, b_sz, n, dim = x.shape
    xs = x.reshape(bt * b_sz, n, dim)
    nc = _get_nc()
    in_maps = [
        {
            "x": np.ascontiguousarray(xs[c]),
            "W_qkv": W_qkv,
            "b_qkv": b_qkv,
            "W_out": W_out,
            "b_out": b_out,
        }
        for c in range(8)
    ]
    res = run_bass_kernel_spmd(nc, in_maps, core_ids=list(range(8)))
    outs = np.stack([np.asarray(res.results[c]["out"]) for c in range(8)])
    return outs.reshape(bt, b_sz, n, dim).astype(np.float32)



# revision 9
# speedup vs baseline: 1.0131x; 1.0131x over previous
"""Multi-head attention block kernel for Trainium2 (8 NeuronCores). v5

Changes vs v2 (151.8us), measured 125.0us loop-slope on this hardware:
  - Q/K projection GEMM in fp8e4 DoubleRow (K=256/instruction, 2x PE
    throughput): W_qkv q|k columns are quantized to fp8 at 2048x in the
    prologue, x^T gets an fp8 shadow copy (DVE), and the 2048^2 descale
    folds into the exp scale with the q/k bias pre-scaled by 2048.
  - attention P@V in fp8e4 DoubleRow over key-block pairs: exp writes
    probabilities directly as fp8 (range [e^-2.5, e^2.5] is all-normal
    in e4m3), v_sb is stored fp8 by the existing PSUM->SBUF copies.
  - scores (K=128: DoubleRow needs K>=256), V projection and the output
    GEMM stay bf16: fp8 there pushes rel_err past the 2e-2 gate.
  - the output GEMM runs one iteration DEFERRED: its 8x8 kh-matmuls
    sprinkle one-per-j through the next iteration's exp-paced phases as
    PE gap filler (the exp stream paces each head phase at ~8.3us vs
    ~7.1us of PE work), and the old 16us ACT-idle tail disappears.  The
    For_i body is emitted twice with outT buffers swapped; an epilogue
    computes the last iteration's output GEMM for real, so the
    single-shot result is exact.
  - x^T fp8 casts and half the v_sb copies run on ACT (idle until the
    first exp) instead of DVE, whose body-start queue otherwise delays
    the paced qk-bias and attention-normalize chains.
"""

import numpy as np

NH = 8
SKIP_ATTN = False
SKIP_EXP = False
AT8 = True
SIMDR = False
JGRAN = 1
PSBIG = 2
PSBANK = 4

P = 128
N_CTX = 1024
DIM = 512
H = 8
HD = 128
QKV = 3072
SCALE = 0.125  # (512 // 8) ** -0.5, faithful to the reference
WS = 2048.0  # W_qkv q|k-column fp8 quantize scale; q,k carry it into bf16
ESCALE = SCALE / (WS * WS)  # exp scale absorbs both 2048 factors

_cached_nc = None


def _build_nc(loop_n=1):
    from contextlib import ExitStack

    import concourse.mybir as mybir
    import concourse.tile as tile
    from concourse import bacc
    from concourse.masks import make_identity

    F32 = mybir.dt.float32
    BF16 = mybir.dt.bfloat16
    FP8 = mybir.dt.float8e4
    DR = mybir.MatmulPerfMode.DoubleRow
    AF = mybir.ActivationFunctionType
    ADD = mybir.AluOpType.add
    MULT = mybir.AluOpType.mult

    nc = bacc.Bacc()

    x_ext = nc.declare_dram_parameter("x", [N_CTX, DIM], F32, isOutput=False)
    wqkv_ext = nc.declare_dram_parameter("W_qkv", [DIM, QKV], F32, isOutput=False)
    bqkv_ext = nc.declare_dram_parameter("b_qkv", [QKV], F32, isOutput=False)
    wout_ext = nc.declare_dram_parameter("W_out", [N_CTX, DIM], F32, isOutput=False)
    bout_ext = nc.declare_dram_parameter("b_out", [DIM], F32, isOutput=False)
    out_ext = nc.declare_dram_parameter("out", [N_CTX, DIM], F32, isOutput=True)

    NT = N_CTX // P  # 8 row tiles
    KD = DIM // P  # 4 contraction chunks for dim=512
    VW = HD + 1  # 129: v columns per head incl. ones column

    with ExitStack() as ctx:
        tc = ctx.enter_context(tile.TileContext(nc))
        consts = ctx.enter_context(tc.tile_pool(name="consts", bufs=1))
        persist = ctx.enter_context(tc.tile_pool(name="persist", bufs=1))
        work = ctx.enter_context(tc.tile_pool(name="work", bufs=2))
        small = ctx.enter_context(tc.tile_pool(name="small", bufs=3))
        ps_big = ctx.enter_context(tc.tile_pool(name="ps_big", bufs=PSBIG, space="PSUM"))
        ps_bank = ctx.enter_context(tc.tile_pool(name="ps_bank", bufs=PSBANK, space="PSUM"))

        # ---- constants / weights (outside any bench loop) -------------------
        ident = consts.tile([P, P], BF16, tag="ident")
        make_identity(nc, ident)
        ones_row = consts.tile([1, P], BF16, tag="ones_row")
        nc.vector.memset(ones_row, 1.0)

        # x via HWDGE fp32 (sync queue), cast + PE-transposed on chip —
        # keeps the gpsimd (SWDGE cast) queue free for the weight loads
        x_sb = persist.tile([P, NT, DIM], F32, tag="x_sb")
        for t in range(NT):
            nc.sync.dma_start(
                x_sb[:, t, :], x_ext.rearrange("(t p) d -> p t d", p=P)[:, t, :]
            )

        # v bias and out bias as single-partition rows (bf16, for the
        # prologue broadcast matmuls).  These go FIRST on the gpsimd queue
        # so the broadcast matmuls (head of the PE stream) unblock fast.
        bv_row = consts.tile([1, H * HD], BF16, tag="bv")
        nc.gpsimd.dma_start(bv_row, bqkv_ext[2 * H * P : QKV][None, :])
        bout_row = consts.tile([1, DIM], BF16, tag="bout")
        nc.gpsimd.dma_start(bout_row, bout_ext[None, :])
        # V columns of W_qkv as (p, ko, 1024) bf16 — the V GEMM runs first.
        wv_sb = consts.tile([P, KD, H * HD], BF16, tag="wv")
        wq_r = wqkv_ext.rearrange("(ko p) n -> p ko n", p=P)
        for k in range(KD):
            nc.gpsimd.dma_start(wv_sb[:, k, :], wq_r[:, k, 2 * H * P :])
        # q|k columns staged f32 per k-chunk through a rotating 2-buf pool
        # (8 KB/partition live instead of 32), DVE-quantized to fp8 at 2048x
        # (Pool's software fp8 convert is ~4x slower than DVE's).
        wqk8 = consts.tile([P, KD, 2 * H * P], FP8, tag="wqk8")
        with tc.tile_pool(name="stage", bufs=2) as stage:
            for k in range(KD):
                wqk_st = stage.tile([P, 2 * H * P], F32, tag="wqk_st")
                nc.sync.dma_start(wqk_st, wq_r[:, k, 0 : 2 * H * P])
                nc.vector.tensor_scalar(wqk8[:, k, :], wqk_st, WS, None, MULT)
        # W_out as (p, kh, 512) bf16 — contraction dim (h*hd) on partitions
        wout_sb = consts.tile([P, H, DIM], BF16, tag="wout")
        nc.gpsimd.dma_start(wout_sb, wout_ext.rearrange("(kh p) c -> p kh c", p=P))
        # q/k bias in partition-major layout: bqk[p, m] = b_qkv[m*128 + p],
        # pre-scaled by 2048 to match the fp8 q/k psum scale
        bqk_st = consts.tile([P, 2 * H], F32, tag="bqk_st")
        nc.sync.dma_start(
            bqk_st, bqkv_ext[0 : 2 * H * P].rearrange("(t p) -> p t", p=P)
        )
        bqk_sb = consts.tile([P, 2 * H], F32, tag="bqk")
        nc.vector.tensor_scalar(bqk_sb, bqk_st, WS, None, MULT)

        # ---- prologue: broadcast bias rows across all 128 partitions -------
        # bc[p, c] = bias[c] via ones-column (K=1) matmuls, once.
        bc_bv = consts.tile([P, H * HD], BF16, tag="bc_bv")
        bc_bout = consts.tile([P, DIM], F32, tag="bc_bout")
        ones_col = consts.tile([1, P], BF16, tag="ones_col")
        nc.vector.memset(ones_col, 1.0)
        for half in range(2):
            bps = ps_bank.tile([P, DIM], F32, tag="bank")
            nc.tensor.matmul(
                bps,
                ones_col,
                bv_row[:, half * DIM : (half + 1) * DIM],
                start=True,
                stop=True,
            )
            nc.vector.tensor_copy(bc_bv[:, half * DIM : (half + 1) * DIM], bps)
        bps = ps_bank.tile([P, DIM], F32, tag="bank")
        nc.tensor.matmul(bps, ones_col, bout_row, start=True, stop=True)
        nc.vector.tensor_copy(bc_bout, bps)

        # out staging lives outside the loop: the last two chunks' DRAM
        # stores are deferred into the NEXT iteration (the For_i drain
        # otherwise idles the PE ~3us waiting on the last store's DMA
        # semaphore); an epilogue after the loop stores them for real.
        out_sb = persist.tile([P, NT, DIM], F32, tag="out_sb")
        out_r = out_ext.rearrange("(t p) c -> p t c", p=P)
        nc.vector.memset(out_sb[:, NT - 2 :, :], 0.0)
        # two outT buffers: the output GEMM for iteration i runs one
        # iteration DEFERRED, sprinkled through iteration i+1's exp-paced
        # phases as PE filler (one kh-matmul per j slot).  The For_i body
        # is emitted twice with the buffers swapped; an epilogue finishes
        # the last iteration's output GEMM for real.
        outT_a = persist.tile([P, H, N_CTX], BF16, tag="outT_a")
        outT_b = persist.tile([P, H, N_CTX], BF16, tag="outT_b")
        nc.vector.memset(outT_a, 0.0)
        nc.vector.memset(outT_b, 0.0)

        def body(outT, outT_prev):
            # ---- x^T: cast to bf16 on Pool, transpose via DMA XBAR on the
            # ACT hwdge ring (separate from the SP ring doing out stores).
            # One batched DMA per row-tile: [128, 512] -> 3D out [128, 4, 128].
            x_bf = work.tile([P, NT, DIM], BF16, tag="x_bf")
            for t in range(NT):
                nc.gpsimd.tensor_copy(x_bf[:, t, :], x_sb[:, t, :])
            xT = work.tile([P, KD, N_CTX], BF16, tag="xT")
            # first two row-tiles transposed on the PE (idle at body start,
            # and ~3us faster than the DMA chain's first-byte latency) so the
            # V GEMM starts immediately; the rest stream via DMA XBAR.
            PE_T = 2
            for t in range(PE_T):
                for c in range(KD):
                    tp = ps_bank.tile([P, P], F32, tag="bank")
                    nc.tensor.matmul(
                        tp,
                        x_bf[:, t, c * P : (c + 1) * P],
                        ident,
                        start=True,
                        stop=True,
                    )
                    nc.vector.tensor_copy(xT[:, c, t * P : (t + 1) * P], tp)
            for t in range(PE_T, NT):
                nc.scalar.dma_start(
                    xT[:, :, t * P : (t + 1) * P],
                    x_bf[:, t, :],
                    transpose=True,
                )
            # fp8 shadow of x^T for the q/k DoubleRow GEMM.  ACT does the
            # convert: it idles until the first exp anyway, while DVE's
            # body-start queue feeds the paced qk-bias/attention chains.
            x8T = work.tile([P, KD, N_CTX], FP8, tag="x8T")
            for t in range(NT):
                nc.scalar.copy(
                    x8T[:, :, t * P : (t + 1) * P], xT[:, :, t * P : (t + 1) * P]
                )

            # ---- v (Form A): n on partitions, heads side by side with a
            # ones column: v_sb[:, t, h*129+128] = 1.0 -> softmax sums ride
            # along in the attention matmul for free.  Emission of the V
            # chunks is deferred into phase 0's j-loop (emit_v below) so the
            # free-running V GEMM fills the PE while exp paces the scores.
            v_sb = work.tile([P, NT, H * VW], FP8 if AT8 else BF16, tag="v_sb")
            nc.vector.memset(
                v_sb.rearrange("p t (h w) -> p t h w", w=VW)[:, :, :, HD : HD + 1],
                1.0,
            )

            def emit_v(t, half):
                ps = ps_bank.tile([P, DIM], F32, tag="bank")
                for k in range(KD):
                    nc.tensor.matmul(
                        ps,
                        xT[:, k, t * P : (t + 1) * P],
                        wv_sb[:, k, half * DIM : (half + 1) * DIM],
                        start=(k == 0),
                        stop=(k == KD - 1),
                    )
                dst = v_sb[:, t, :].rearrange("p (h w) -> p h w", w=VW)[
                    :, half * 4 : (half + 1) * 4, 0:HD
                ]
                src = ps.rearrange("p (h w) -> p h w", w=HD)
                # v stays UNbiased: P@(v+bv)/d == (P@v)/d + bv, so bv is
                # added during the per-query normalize instead.  Half the
                # copies go to ACT to unclog DVE's phase-0 queue.
                if half == 0:
                    nc.vector.tensor_copy(dst, src)
                else:
                    nc.scalar.copy(dst, src)

            emit_v(0, 0)
            emit_v(0, 1)

            # ---- per-head software pipeline --------------------------------
            # Engines execute their scheduled streams in-order, so the
            # EMISSION order is the schedule.  Interleave head h's scores
            # (whose PSUM slots recycle at ScalarE's exp pace) with head
            # h-1's attention matmuls so the PE never waits inline on exp;
            # the final GEMM interleaves with the last head's attention.
            def emit_qk(h):
                pair = []
                for part in range(2):  # 0: q, 1: k
                    m = part * H + h
                    qk = work.tile([P, N_CTX], BF16, tag=f"qkT{part}")
                    for half in range(2):
                        sl = slice(half * DIM, (half + 1) * DIM)
                        ps = ps_bank.tile([P, DIM], F32, tag="bank")
                        for kp in range(2):
                            if SIMDR:
                                nc.tensor.matmul(
                                    ps,
                                    wqk8[:, 2 * kp, m * P : (m + 1) * P],
                                    x8T[:, 2 * kp, sl],
                                    start=(kp == 0),
                                    stop=(kp == 1),
                                )
                            else:
                                nc.tensor.matmul(
                                    ps,
                                    wqk8[:, 2 * kp : 2 * kp + 2, m * P : (m + 1) * P],
                                    x8T[:, 2 * kp : 2 * kp + 2, sl],
                                    start=(kp == 0),
                                    stop=(kp == 1),
                                    perf_mode=DR,
                                )
                        nc.vector.tensor_scalar_add(
                            qk[:, sl], ps, bqk_sb[:, m : m + 1]
                        )
                    pair.append(qk)
                return pair

            def emit_scores_j(qkT_pair, pT, j):
                qT_h, kT_h = qkT_pair
                ps = ps_big.tile([P, N_CTX], F32, tag="big")
                for half in range(2):
                    sl = slice(half * DIM, (half + 1) * DIM)
                    nc.tensor.matmul(
                        ps[:, sl],
                        kT_h[:, j * P : (j + 1) * P],
                        qT_h[:, sl],
                        start=True,
                        stop=True,
                    )
                if not SKIP_EXP:
                    nc.scalar.activation(pT[:, j, :], ps, AF.Exp, scale=ESCALE)
                else:
                    nc.vector.tensor_copy(pT[:, j, 0:4], ps[:, 0:4])

            def emit_attn_ic(h, pT, ic, at_stage):
                if SKIP_ATTN:
                    nc.vector.memset(at_stage[:, ic, :], 0.5)
                    return
                aps = ps_bank.tile([P, VW], F32, tag="bank")
                if AT8 and SIMDR:
                    for j in range(0, NT, 2):
                        nc.tensor.matmul(
                            aps[:, :VW],
                            pT[:, j, ic * P : (ic + 1) * P],
                            v_sb[:, j, h * VW : (h + 1) * VW],
                            start=(j == 0),
                            stop=(j == NT - 2),
                        )
                elif AT8:
                    for j in range(0, NT, 2):
                        nc.tensor.matmul(
                            aps[:, :VW],
                            pT[:, j : j + 2, ic * P : (ic + 1) * P],
                            v_sb[:, j : j + 2, h * VW : (h + 1) * VW],
                            start=(j == 0),
                            stop=(j == NT - 2),
                            perf_mode=DR,
                        )
                else:
                    for j in range(NT):
                        nc.tensor.matmul(
                            aps[:, :VW],
                            pT[:, j, ic * P : (ic + 1) * P],
                            v_sb[:, j, h * VW : (h + 1) * VW],
                            start=(j == 0),
                            stop=(j == NT - 1),
                        )
                rc = small.tile([P, 1], F32, tag="rc")
                nc.vector.reciprocal(rc, aps[:, HD : HD + 1])
                # at = (P@v)*1/d + bv  (fused normalize + v-bias)
                nc.vector.scalar_tensor_tensor(
                    at_stage[:, ic, :],
                    aps[:, 0:HD],
                    rc,
                    bc_bv[:, h * HD : (h + 1) * HD],
                    op0=MULT,
                    op1=ADD,
                )

            def emit_transp_dma(h, at_stage):
                # one batched XBAR transpose for the whole head, on the SP
                # HWDGE ring: a 667ns queue op on the ACT sequencer would
                # sit between exp activations and stall the paced stream.
                nc.sync.dma_start(
                    outT[:, h, :].rearrange("p (g q) -> p g q", q=P),
                    at_stage,
                    transpose=True,
                )

            def emit_transp_pe(h, ic, at_stage):
                tp = ps_bank.tile([P, P], F32, tag="bank")
                nc.tensor.matmul(
                    tp, at_stage[:, ic, :], ident, start=True, stop=True
                )
                nc.vector.tensor_copy(outT[:, h, ic * P : (ic + 1) * P], tp)

            fin_state = {}

            def emit_final_kh(ic, kh):
                # one kh-step of the deferred output GEMM for chunk ic of
                # the PREVIOUS iteration; the psum bank stays live across
                # the phase's j-loop and drains via stt+store at kh==H-1.
                if kh == 0:
                    fps_new = ps_bank.tile([P, DIM], F32, tag="bank")
                    fin_state[ic] = fps_new
                fps = fin_state[ic]
                nc.tensor.matmul(
                    fps,
                    outT_prev[:, kh, ic * P : (ic + 1) * P],
                    wout_sb[:, kh, :],
                    start=(kh == 0),
                    stop=(kh == H - 1),
                )
                if kh == H - 1:
                    # out = fps*1 + b_out  (fused copy + bias on DVE)
                    nc.vector.scalar_tensor_tensor(
                        out_sb[:, ic, :], fps, 1.0, bc_bout, op0=MULT, op1=ADD
                    )
                    nc.sync.dma_start(out_r[:, ic, :], out_sb[:, ic, :])

            # qk for head h is emitted mid-phase h-1 (right after scores
            # j=2): its DVE bias adds drain ahead of the attention stt
            # backlog, so the next head's scores (and the paced exp stream)
            # start without the ~2us head-boundary DVE latency bubble.
            qk_use = emit_qk(0)
            pT_prev = work.tile([P, NT, N_CTX], FP8 if AT8 else BF16, tag="pT")
            v_rest = [(t, half) for t in range(1, NT) for half in range(2)]
            qk_next = None
            for j in range(NT):
                emit_scores_j(qk_use, pT_prev, j)
                if j == 2 and NH > 1:
                    qk_next = emit_qk(1)
                # V GEMM chunks fill the exp-paced gaps of phase 0
                for _ in range(2):
                    if v_rest:
                        emit_v(*v_rest.pop(0))
            for h in range(1, NH + 1):
                hp = h - 1  # head whose attention runs this phase
                if h < NH:
                    qk_cur, qk_next = qk_next, None
                    pT_cur = work.tile([P, NT, N_CTX], FP8 if AT8 else BF16, tag="pT")
                at_stage = work.tile([P, NT, P], BF16, tag="at_stage")
                for j in range(NT):
                    if h < NH:
                        emit_scores_j(qk_cur, pT_cur, j)
                        if j == 2 and h + 1 < NH:
                            qk_next = emit_qk(h + 1)
                    # attention of the previous head fills the exp latency
                    emit_attn_ic(hp, pT_prev, j, at_stage)
                    # one kh-step of the deferred output GEMM per j slot
                    emit_final_kh(h - 1, j)
                # whole-head batched XBAR transpose — with the output GEMM
                # deferred a full iteration, even the last head's outT isn't
                # needed until the next body, so the DMA path (instead of
                # PE matmuls + DVE copies) works for all 8 heads.
                emit_transp_dma(hp, at_stage)
                if h < NH:
                    pT_prev = pT_cur

        def epilogue(outT_last):
            # the last iteration's output GEMM, done for real
            for ic in range(NT):
                fps = ps_bank.tile([P, DIM], F32, tag="bank")
                for kh in range(H):
                    nc.tensor.matmul(
                        fps,
                        outT_last[:, kh, ic * P : (ic + 1) * P],
                        wout_sb[:, kh, :],
                        start=(kh == 0),
                        stop=(kh == H - 1),
                    )
                nc.vector.scalar_tensor_tensor(
                    out_sb[:, ic, :], fps, 1.0, bc_bout, op0=MULT, op1=ADD
                )
                nc.sync.dma_start(out_r[:, ic, :], out_sb[:, ic, :])

        if loop_n == 1:
            body(outT_a, outT_b)
        else:
            with tc.For_i(0, max(1, loop_n // 2), 1):
                body(outT_a, outT_b)
                body(outT_b, outT_a)
        epilogue(outT_a)

    nc.finalize()
    return nc


def _get_nc():
    global _cached_nc
    if _cached_nc is None:
        _cached_nc = _build_nc()
    return _cached_nc


def kernel(**inputs):
    from concourse.bass_utils import run_bass_kernel_spmd

    x = np.ascontiguousarray(np.asarray(inputs["x"], dtype=np.float32))
    W_qkv = np.ascontiguousarray(np.asarray(inputs["W_qkv"], dtype=np.float32))
    b_qkv = np.ascontiguousarray(np.asarray(inputs["b_qkv"], dtype=np.float32))
    W_out = np.ascontiguousarray(np.asarray(inputs["W_out"], dtype=np.float32))
    b_out = np.ascontiguousarray(np.asarray(inputs["b_out"], dtype=np.float32))

    bt, b_sz, n, dim = x.shape
    xs = x.reshape(bt * b_sz, n, dim)
    nc = _get_nc()
    in_maps = [
        {
            "x": np.ascontiguousarray(xs[c]),
            "W_qkv": W_qkv,
            "b_qkv": b_qkv,
            "W_out": W_out,
            "b_out": b_out,
        }
        for c in range(8)
    ]
    res = run_bass_kernel_spmd(nc, in_maps, core_ids=list(range(8)))
    outs = np.stack([np.asarray(res.results[c]["out"]) for c in range(8)])
    return outs.reshape(bt, b_sz, n, dim).astype(np.float32)

